# revision 1
# baseline (speedup 1.0000x reference)
"""Self-contained Trainium2 Bass kernel for nn_AutoRegressive_88837103551116.

2-layer LSTM (HID=64) over ragged sequences: warmup pass over x (per-sample
lengths), then autoregressive decode over [dense(h_top_final), context_t].
Pure data-parallel over 8 NeuronCores (batch 512 -> 64 per core).

Device algorithm (per core):
  - slot s computes layer0 @ step s and layer1 @ step s-1 (layer stagger) so
    both layers' gates share each tick's instructions
  - states [feature, batch]: rb [128,B]=[h0;h1], cc [64,2B]=[c0|c1]
  - gates via 8 small matmuls/tick into 2 PSUM banks (one accumulation group
    open per bank at a time; recurrence-independent matmuls lead each tick so
    the PE runs ahead); biases and the ragged-sequence c-freeze (+/-BIG added
    to i/f pre-activations past each sample's length) are folded into extra
    stationary-weight rows
  - h at the last valid step is captured into hkeep via copy_predicated with
    uint8 mask streams (off the recurrence critical path)
  - host side: input transposes/stream building, output -999 masking
"""
import sys

import numpy as np

try:
    import concourse.bass as bass
except ImportError:
    sys.path.insert(0, "/opt/trn_rl_repo")
    import concourse.bass as bass

import contextlib
import json

import concourse.tile as tile
from concourse import mybir
from concourse.bass_utils import run_bass_kernel_spmd

N_CORES = 8
TW = 512
TC = 512



H = 64
IN = 16
F = 8
C = 8
BIG = 50.0


def build_weights(Wih0, Whh0, bih0, bhh0, Wih1, Whh1, bih1, bhh1, Wd, bd):
    """Build all stationary lhsT matrices (shared across cores). fp32."""
    b0 = bih0 + bhh0
    b1 = bih1 + bhh1
    maskcol_if = np.concatenate([np.full(H, -BIG), np.full(H, BIG)]).astype(np.float32)

    def stack_l0(gate_rows, xw, bias, mask):
        # lhsT [18, 128]: rows 0:16 = xw^T, 16 = mask, 17 = bias
        out = np.zeros((18, 128), np.float32)
        out[0:xw.shape[1], :] = xw[gate_rows].T
        out[16] = mask
        out[17] = bias[gate_rows]
        return out

    gi = slice(0, 128)   # i,f rows
    gg = slice(128, 256)  # g,o rows
    W = {}
    W["w0x_if"] = stack_l0(gi, Wih0, b0, maskcol_if)
    W["w0x_go"] = stack_l0(gg, Wih0, b0, np.zeros(128, np.float32))
    W["w0h_if"] = Whh0[gi].T.copy()   # [64,128]
    W["w0h_go"] = Whh0[gg].T.copy()
    W["w1_if"] = np.concatenate([Wih1[gi].T, Whh1[gi].T], 0)  # [128,128]
    W["w1_go"] = np.concatenate([Wih1[gg].T, Whh1[gg].T], 0)
    W["wc_if"] = np.stack([b1[gi], maskcol_if]).astype(np.float32)  # [2,128]
    W["wc_go"] = b1[gg].reshape(1, 128).astype(np.float32)
    # decode l0: input rows 0:8 = ctx weights (cols 8:16 of Wih0), elem separate
    def stack_l0_dec(gate_rows, mask):
        out = np.zeros((18, 128), np.float32)
        out[0:8, :] = Wih0[gate_rows, 8:16].T
        out[16] = mask
        out[17] = b0[gate_rows]
        return out
    W["d0x_if"] = stack_l0_dec(gi, maskcol_if)
    W["d0x_go"] = stack_l0_dec(gg, np.zeros(128, np.float32))
    W["d0e_if"] = Wih0[gi, 0:8].T.copy()  # [8,128]
    W["d0e_go"] = Wih0[gg, 0:8].T.copy()
    W["wdT"] = Wd.T.copy()  # [64,8]
    W["bd"] = bd.reshape(8, 1).copy()
    for k in W:
        W[k] = np.ascontiguousarray(W[k], np.float32)
    return W


def build_streams(x, lengths_x, context, lengths_ctx):
    """Per-core streams. x [B,TW,16], context [B,TC,8]. B arbitrary."""
    B = x.shape[0]
    TW = x.shape[1]
    TC = context.shape[1]
    SW, SD = 528, 512
    CW, CD = SW // 8, SD // 8

    s_idx = np.arange(SW)
    mw = (s_idx[:, None] < lengths_x[None, :]).astype(np.float32)  # [SW,B]
    mw1 = np.zeros_like(mw)
    mw1[1:] = mw[:-1]

    WA = np.zeros((CW, 18, 8, B), np.float32)
    xt = np.transpose(x, (1, 2, 0))  # [TW,16,B]
    WA[:, 0:16].reshape(CW * 8 * 16, B)
    WA_r = WA.reshape(CW, 18, 8, B)
    for c in range(CW):
        for t in range(8):
            s = c * 8 + t
            if s < TW:
                WA_r[c, 0:16, t, :] = xt[s]
            WA_r[c, 16, t, :] = 1.0 - mw[s]
            WA_r[c, 17, t, :] = 1.0
    WC = np.zeros((CW, 2, 8, B), np.float32)
    WC[:, 0] = 1.0
    WC[:, 1] = (1.0 - mw1).reshape(CW, 8, B)
    NMw = np.zeros((CW, 128, 8, B), np.uint8)
    NMw[:, 0:64] = mw.reshape(CW, 8, 1, B).transpose(0, 2, 1, 3)
    NMw[:, 64:128] = mw1.reshape(CW, 8, 1, B).transpose(0, 2, 1, 3)

    md0 = np.zeros((SD, B), np.float32)
    md0[0:TC - 1] = 1.0     # l0 steps 0..510 active; 511 pad frozen
    md1 = np.ones((SD, B), np.float32)
    md1[0] = 0.0            # freeze l1 at slot 0
    DA = np.zeros((CD, 18, 8, B), np.float32)
    ctxt = np.transpose(context, (1, 2, 0))  # [TC,8,B]
    DA_r = DA
    for c in range(CD):
        for t in range(8):
            s = c * 8 + t
            if s < TC - 1:
                DA_r[c, 0:8, t, :] = ctxt[s]
            DA_r[c, 16, t, :] = 1.0 - md0[s]
            DA_r[c, 17, t, :] = 1.0
    DC = np.zeros((CD, 2, 8, B), np.float32)
    DC[:, 0] = 1.0
    DC[:, 1] = (1.0 - md1).reshape(CD, 8, B)
    NMd = np.zeros((CD, 128, 8, B), np.uint8)
    NMd[:, 0:64] = (1.0 - md0).reshape(CD, 8, 1, B).transpose(0, 2, 1, 3)
    NMd[:, 64:128] = (1.0 - md1).reshape(CD, 8, 1, B).transpose(0, 2, 1, 3)

    def pad1(a):
        return np.concatenate([a, np.zeros_like(a[:1])], 0)
    return dict(
        wa=pad1(WA.reshape(CW, 18, 8 * B)),
        wcs=pad1(WC.reshape(CW, 2, 8 * B)),
        nmw=pad1(NMw.reshape(CW, 128, 8 * B)),
        da=pad1(DA.reshape(CD, 18, 8 * B)),
        dcs=pad1(DC.reshape(CD, 2, 8 * B)),
        nmd=pad1(NMd.reshape(CD, 128, 8 * B)),
    )


def post_outputs(YE, YD, bd, lengths_ctx, TC):
    """YE [8,B], YD [CD,8,8,B] -> out [B,TC,8] with -999 padding."""
    B = YE.shape[1]
    out = np.zeros((B, TC, F), np.float32)
    out[:, 0, :] = YE.T
    ysd = YD.transpose(0, 2, 1, 3).reshape(512, F, B)  # [slot, F, B]
    # ys_t = slot t+1 for t = 0..510
    out[:, 1:TC, :] = ysd[1:TC].transpose(2, 0, 1) + bd[None, None, :]
    valid = np.arange(TC)[None, :] < lengths_ctx[:, None]
    return np.where(valid[:, :, None], out, np.float32(-999.0))



import contextlib

import concourse.bass as bass
import concourse.tile as tile
from concourse import mybir

F32 = mybir.dt.float32
U8 = mybir.dt.uint8
AF = mybir.ActivationFunctionType

B = 64
H = 64
SW = 528   # warmup slots (padded; needs >= 513)
SD = 512   # decode slots (l0 steps 0..510 + 1 pad)
CW = SW // 8
CD = SD // 8


def build_nc(repeat=1, static=False):
    nc = bass.Bass("TRN2", target_bir_lowering=False, debug=False)
    dt = F32

    d = {}
    d["wa"] = nc.dram_tensor("wa", [CW + 1, 18, 512], dt, kind="ExternalInput")
    d["wcs"] = nc.dram_tensor("wcs", [CW + 1, 2, 512], dt, kind="ExternalInput")
    d["nmw"] = nc.dram_tensor("nmw", [CW + 1, 128, 512], U8, kind="ExternalInput")
    d["da"] = nc.dram_tensor("da", [CD + 1, 18, 512], dt, kind="ExternalInput")
    d["dcs"] = nc.dram_tensor("dcs", [CD + 1, 2, 512], dt, kind="ExternalInput")
    for name, shp in [
        ("w0x_if", [18, 128]), ("w0x_go", [18, 128]),
        ("w0h_if", [64, 128]), ("w0h_go", [64, 128]),
        ("w1_if", [128, 128]), ("w1_go", [128, 128]),
        ("wc_if", [2, 128]), ("wc_go", [1, 128]),
        ("d0x_if", [18, 128]), ("d0x_go", [18, 128]),
        ("d0e_if", [8, 128]), ("d0e_go", [8, 128]),
        ("wdT", [64, 8]), ("bd", [8, 1]),
    ]:
        d[name] = nc.dram_tensor(name, shp, dt, kind="ExternalInput")
    ye = nc.dram_tensor("ye", [8, B], dt, kind="ExternalOutput")
    yd = nc.dram_tensor("yd", [CD, 8, 512], dt, kind="ExternalOutput")

    with tile.TileContext(nc) as tc:
        with (
            tc.tile_pool(name="consts", bufs=1) as consts,
            tc.tile_pool(name="state", bufs=1) as state,
            tc.tile_pool(name="stream", bufs=1) as stream,
            tc.tile_pool(name="work", bufs=3) as work,
            tc.tile_pool(name="psum", bufs=2, space="PSUM") as psum,
            tc.tile_pool(name="outp", bufs=1, space="PSUM") as outp,
        ):
            W = {}
            for name in ["w0x_if", "w0x_go", "w0h_if", "w0h_go", "w1_if",
                         "w1_go", "wc_if", "wc_go", "d0x_if", "d0x_go",
                         "d0e_if", "d0e_go", "bd"]:
                t = consts.tile(list(d[name].shape), dt, tag=name, name="w_" + name)
                nc.sync.dma_start(out=t, in_=d[name][:, :])
                W[name] = t
            wdT_t = consts.tile([128, 8], dt, tag="wdT", name="w_wdT")
            nc.sync.dma_start(out=wdT_t[64:128, :], in_=d["wdT"][:, :])
            W["wdT"] = wdT_t

            rb = [state.tile([128, B], dt, tag=f"rb{i}", name=f"rb{i}") for i in range(2)]
            cc = [state.tile([64, 2 * B], dt, tag=f"cc{i}", name=f"cc{i}") for i in range(2)]
            for i in range(2):
                nc.vector.memset(rb[i], 0.0)
                nc.vector.memset(cc[i], 0.0)

            saA = stream.tile([18, 512], dt, tag="saA")
            saB = stream.tile([18, 512], dt, tag="saB")
            scA = stream.tile([2, 512], dt, tag="scA")
            scB = stream.tile([2, 512], dt, tag="scB")
            nmA = stream.tile([128, 512], U8, tag="nmA")
            nmB = stream.tile([128, 512], U8, tag="nmB")
            elem = state.tile([8, B], dt, tag="elem")
            hkeep = state.tile([128, B], dt, tag="hkeep")
            nc.vector.memset(hkeep, 0.0)

            def tick(sl, sa, sc, nm, decode):
                par = sl % 2
                rbp, rbn = rb[par], rb[1 - par]
                ccp, ccn_dst = cc[par], cc[1 - par]
                t8 = sl % 8
                colB = slice(t8 * B, (t8 + 1) * B)
                megaIF = psum.tile([128, 2 * B], dt, tag="megaIF", name="megaIF")
                megaGO = psum.tile([128, 2 * B], dt, tag="megaGO", name="megaGO")

                wx_if = W["d0x_if"] if decode else W["w0x_if"]
                wx_go = W["d0x_go"] if decode else W["w0x_go"]

                # Gate matmuls. Two PSUM banks (IF / GO), at most one open
                # accumulation group per bank; the rbp-independent leading MMs
                # let the PE run ahead during the previous tick's tail.
                if decode:
                    nc.tensor.matmul(megaIF[:, 0:B], W["d0e_if"], elem, start=True, stop=False)
                    nc.tensor.matmul(megaGO[:, 0:B], W["d0e_go"], elem, start=True, stop=False)
                    nc.tensor.matmul(megaIF[:, 0:B], wx_if, sa[0:18, colB], start=False, stop=False)
                    nc.tensor.matmul(megaGO[:, 0:B], wx_go, sa[0:18, colB], start=False, stop=False)
                else:
                    nc.tensor.matmul(megaIF[:, 0:B], wx_if, sa[0:18, colB], start=True, stop=False)
                    nc.tensor.matmul(megaGO[:, 0:B], wx_go, sa[0:18, colB], start=True, stop=False)
                nc.tensor.matmul(megaIF[:, 0:B], W["w0h_if"], rbp[0:64, :], start=False, stop=True)
                nc.tensor.matmul(megaIF[:, B:2 * B], W["wc_if"], sc[0:2, colB], start=True, stop=False)
                nc.tensor.matmul(megaIF[:, B:2 * B], W["w1_if"], rbp[:, :], start=False, stop=True)
                nc.tensor.matmul(megaGO[:, 0:B], W["w0h_go"], rbp[0:64, :], start=False, stop=True)
                nc.tensor.matmul(megaGO[:, B:2 * B], W["wc_go"], sc[0:1, colB], start=True, stop=False)
                nc.tensor.matmul(megaGO[:, B:2 * B], W["w1_go"], rbp[:, :], start=False, stop=True)

                # activations (sif in PSUM: exempts t1/t2 from equal-base rule)
                sif = psum.tile([128, 2 * B], dt, tag="sif", name="sif")
                tg = work.tile([64, 2 * B], dt, tag="tg", name="tg")
                so = work.tile([64, 2 * B], dt, tag="so", name="so")
                nc.scalar.activation(sif, megaIF[:, :], AF.Sigmoid)
                nc.scalar.activation(tg, megaGO[0:64, :], AF.Tanh)
                nc.scalar.activation(so, megaGO[64:128, :], AF.Sigmoid)

                # elementwise
                t1 = work.tile([64, 2 * B], dt, tag="t1", name="t1")
                t2 = work.tile([64, 2 * B], dt, tag="t2", name="t2")
                th = work.tile([64, 2 * B], dt, tag="th", name="th")
                nc.vector.tensor_mul(t1, sif[0:64, :], tg)
                nc.vector.tensor_mul(t2, sif[64:128, :], ccp)
                nc.vector.tensor_add(ccn_dst, t1, t2)
                nc.scalar.activation(th, ccn_dst, AF.Tanh)
                nc.vector.tensor_mul(rbn[0:64, :], so[:, 0:B], th[:, 0:B])
                nc.gpsimd.tensor_mul(rbn[64:128, :], so[:, B:2 * B], th[:, B:2 * B])

                if nm is not None:
                    # capture h at each sample's last active slot (off the
                    # recurrence critical path)
                    nc.vector.copy_predicated(hkeep, nm[:, colB], rbn)
                return rbn

            def loop(n):
                # static: python-unrolled; else: hardware For_i
                if static:
                    return contextlib.nullcontext(enumerate(range(n)))
                return None

            rep_cm = tc.For_i(0, repeat, 1) if repeat > 1 else contextlib.nullcontext()
            with rep_cm:
                # ================= warmup =================
                nc.sync.dma_start(out=saA, in_=d["wa"][0, :, :])
                nc.sync.dma_start(out=scA, in_=d["wcs"][0, :, :])
                nc.sync.dma_start(out=nmA, in_=d["nmw"][0, :, :])
                def warm_body(j, i1, i2, first=False):
                    nc.sync.dma_start(out=saB, in_=d["wa"][i1, :, :])
                    nc.sync.dma_start(out=scB, in_=d["wcs"][i1, :, :])
                    nc.sync.dma_start(out=nmB, in_=d["nmw"][i1, :, :])
                    for sl in range(8):
                        tick(sl, saA, scA, nmA, False)
                        if first and sl == 0:
                            nc.vector.memset(rb[1][64:128, :], 0.0)
                    nc.sync.dma_start(out=saA, in_=d["wa"][i2, :, :])
                    nc.sync.dma_start(out=scA, in_=d["wcs"][i2, :, :])
                    nc.sync.dma_start(out=nmA, in_=d["nmw"][i2, :, :])
                    for sl in range(8, 16):
                        tick(sl, saB, scB, nmB, False)

                if static:
                    for j in range(CW // 2):
                        warm_body(j, j * 2 + 1, j * 2 + 2, first=(j == 0))
                else:
                    warm_body(0, 1, 2, first=True)
                    with tc.For_i(1, CW // 2, 1, hint_engines=(mybir.EngineType.PE,)) as j:
                        warm_body(j, nc.snap(j * 2 + 1), nc.snap(j * 2 + 2))

                # ================= elem =================
                nc.vector.tensor_copy(rb[0], hkeep)
                pe = outp.tile([8, B], dt, tag="ops", name="pe")
                nc.tensor.matmul(pe, W["wdT"][64:128, :], rb[0][64:128, :], start=True, stop=True)
                nc.scalar.activation(elem, pe, AF.Identity, bias=W["bd"][:, 0:1])
                nc.sync.dma_start(out=ye[:, :], in_=elem)

                # ================= decode =================
                nc.sync.dma_start(out=saA, in_=d["da"][0, :, :])
                nc.sync.dma_start(out=scA, in_=d["dcs"][0, :, :])
                def dec_body(j, i0, i1, i2, first=False):
                    nc.sync.dma_start(out=saB, in_=d["da"][i1, :, :])
                    nc.sync.dma_start(out=scB, in_=d["dcs"][i1, :, :])
                    ops = outp.tile([8, 512], dt, tag="ops", name="ops")
                    for sl in range(8):
                        rbn = tick(sl, saA, scA, None, True)
                        if first and sl == 0:
                            nc.vector.tensor_copy(rb[1][64:128, :], rb[0][64:128, :])
                        nc.tensor.matmul(ops[:, sl * B:(sl + 1) * B], W["wdT"][64:128, :],
                                         rbn[64:128, :], start=True, stop=True)
                    oso = work.tile([8, 512], dt, tag="oso", name="oso")
                    nc.scalar.copy(oso, ops)
                    nc.sync.dma_start(out=yd[i0, :, :], in_=oso)
                    nc.sync.dma_start(out=saA, in_=d["da"][i2, :, :])
                    nc.sync.dma_start(out=scA, in_=d["dcs"][i2, :, :])
                    ops2 = outp.tile([8, 512], dt, tag="ops2", name="ops2")
                    for sl in range(8, 16):
                        rbn = tick(sl, saB, scB, None, True)
                        nc.tensor.matmul(ops2[:, (sl - 8) * B:(sl - 7) * B], W["wdT"][64:128, :],
                                         rbn[64:128, :], start=True, stop=True)
                    oso2 = work.tile([8, 512], dt, tag="oso2", name="oso2")
                    nc.scalar.copy(oso2, ops2)
                    nc.sync.dma_start(out=yd[i1, :, :], in_=oso2)

                if static:
                    for j in range(CD // 2):
                        dec_body(j, j * 2, j * 2 + 1, j * 2 + 2, first=(j == 0))
                else:
                    dec_body(0, 0, 1, 2, first=True)
                    with tc.For_i(1, CD // 2, 1, hint_engines=(mybir.EngineType.PE,)) as j:
                        dec_body(j, nc.snap(j * 2), nc.snap(j * 2 + 1), nc.snap(j * 2 + 2))

    return nc


def legalize_waits(nc, max_waits=1):
    """walrus codegen caps semaphore waits per instruction; move extras onto
    NoOp instructions inserted immediately before (same engine)."""
    j = json.loads(mybir.module_to_json_bytes(nc.m))
    for fn in j.get("functions", []):
        for blk in fn.get("blocks", []):
            out = []
            for inst in blk.get("instructions", []):
                si = inst.get("sync_info") or {}
                waits = si.get("on_wait") or []
                if len(waits) > max_waits:
                    keep, extra = waits[-max_waits:], waits[:-max_waits]
                    for k, w in enumerate(extra):
                        out.append({"name": f"{inst['name']}-wsp{k}",
                                    "opcode": "NoOp", "engine": inst["engine"],
                                    "ins": [], "outs": [],
                                    "sync_info": {"on_wait": [w], "on_update": []}})
                    si = dict(si); si["on_wait"] = keep
                    inst = dict(inst); inst["sync_info"] = si
                out.append(inst)
            blk["instructions"] = out
    nc.m = mybir.module_from_json_bytes(json.dumps(j).encode())
    return nc


_NC_CACHE = {}


def _get_nc(repeat=1):
    if repeat not in _NC_CACHE:
        nc = build_nc(repeat)
        legalize_waits(nc)
        _NC_CACHE[repeat] = nc
    return _NC_CACHE[repeat]


def build_in_maps(x, lengths_x, context, lengths_ctx,
                  Wih0, Whh0, bih0, bhh0, Wih1, Whh1, bih1, bhh1, Wd, bd):
    Wt = build_weights(Wih0.astype(np.float32), Whh0.astype(np.float32),
                       bih0.astype(np.float32), bhh0.astype(np.float32),
                       Wih1.astype(np.float32), Whh1.astype(np.float32),
                       bih1.astype(np.float32), bhh1.astype(np.float32),
                       Wd.astype(np.float32), bd.astype(np.float32))
    Bn = x.shape[0] // N_CORES
    in_maps = []
    for core in range(N_CORES):
        sl = slice(core * Bn, (core + 1) * Bn)
        st = build_streams(np.ascontiguousarray(x[sl], dtype=np.float32),
                           np.asarray(lengths_x[sl], dtype=np.int64),
                           np.ascontiguousarray(context[sl], dtype=np.float32),
                           np.asarray(lengths_ctx[sl], dtype=np.int64))
        m = dict(st)
        m.pop("nmd", None)
        m.update(Wt)
        in_maps.append(m)
    return in_maps


def kernel(x, lengths_x, context, lengths_ctx,
           Wih0, Whh0, bih0, bhh0, Wih1, Whh1, bih1, bhh1, Wd, bd):
    x = np.asarray(x)
    context = np.asarray(context)
    lengths_x = np.asarray(lengths_x)
    lengths_ctx = np.asarray(lengths_ctx)
    in_maps = build_in_maps(x, lengths_x, context, lengths_ctx,
                            np.asarray(Wih0), np.asarray(Whh0), np.asarray(bih0),
                            np.asarray(bhh0), np.asarray(Wih1), np.asarray(Whh1),
                            np.asarray(bih1), np.asarray(bhh1), np.asarray(Wd),
                            np.asarray(bd))
    nc = _get_nc(1)
    res = run_bass_kernel_spmd(nc, in_maps, core_ids=list(range(N_CORES)))
    Bn = x.shape[0] // N_CORES
    outs = []
    bd32 = np.asarray(bd, dtype=np.float32)
    for core in range(N_CORES):
        sl = slice(core * Bn, (core + 1) * Bn)
        YE = res.results[core]["ye"]
        YD = res.results[core]["yd"].reshape(CD, 8, 8, Bn)
        outs.append(post_outputs(YE, YD, bd32,
                                 np.asarray(lengths_ctx[sl], dtype=np.int64), TC))
    return np.concatenate(outs, axis=0).astype(np.float32)



# revision 2
# speedup vs baseline: 1.6670x; 1.6670x over previous
"""Self-contained Trainium2 Bass kernel for nn_AutoRegressive_88837103551116.

v3: bf16 matmuls; single PSUM accumulation bank per tick [128, 4B]
(cols 0:2B = [f;i] gates for [l0|l1], cols 2B:4B = [o;g]); merged
x+c-stream matmul (6 matmuls/tick warmup, 8 decode); V=sif*U in PSUM so
the partition fold is verifier-legal; c state at U[0:64]; both h-output
muls on DVE.

2-layer LSTM (HID=64) over ragged sequences: warmup pass over x (per-sample
lengths), then autoregressive decode over [dense(h_top_final), context_t].
Pure data-parallel over 8 NeuronCores (batch 512 -> 64 per core).
"""
import sys

import numpy as np
import ml_dtypes

BF = ml_dtypes.bfloat16

try:
    import concourse.bass as bass
except ImportError:
    sys.path.insert(0, "/opt/trn_rl_repo")
    import concourse.bass as bass

import contextlib
import json

import concourse.tile as tile
from concourse import mybir
from concourse.bass_utils import run_bass_kernel_spmd

N_CORES = 8
TW = 512
TC = 512


H = 64
IN = 16
F = 8
C = 8
BIG = 50.0

# gate-row reorderings: IF bank packs [f; i], GO bank packs [o; g]
FI = np.concatenate([np.arange(64, 128), np.arange(0, 64)])
OG = np.concatenate([np.arange(192, 256), np.arange(128, 192)])
MASKCOL = np.concatenate([np.full(64, BIG), np.full(64, -BIG)]).astype(np.float32)


def build_weights(Wih0, Whh0, bih0, bhh0, Wih1, Whh1, bih1, bhh1, Wd, bd):
    """Build all stationary lhsT matrices (shared across cores). bf16."""
    b0 = bih0 + bhh0
    b1 = bih1 + bhh1

    def stack_xc(idx, xw_cols, mask_on):
        # lhsT [20, 128]: rows 0:16 x-part weights, 16 l0 mask, 17 l0 bias,
        # 18 l1 bias, 19 l1 mask
        out = np.zeros((20, 128), np.float32)
        out[0:xw_cols.shape[1], :] = xw_cols[idx].T
        if mask_on:
            out[16] = MASKCOL
            out[19] = MASKCOL
        out[17] = b0[idx]
        out[18] = b1[idx]
        return out

    W = {}
    W["xc_if"] = stack_xc(FI, Wih0, True)
    W["xc_go"] = stack_xc(OG, Wih0, False)
    W["w0h_if"] = Whh0[FI].T.copy()   # [64,128]
    W["w0h_go"] = Whh0[OG].T.copy()
    W["w1_if"] = np.concatenate([Wih1[FI].T, Whh1[FI].T], 0)  # [128,128]
    W["w1_go"] = np.concatenate([Wih1[OG].T, Whh1[OG].T], 0)
    # decode: ctx cols 8:16 of Wih0; elem cols 0:8 separate
    def stack_xc_dec(idx, mask_on):
        out = np.zeros((20, 128), np.float32)
        out[0:8, :] = Wih0[idx, 8:16].T
        if mask_on:
            out[16] = MASKCOL
            out[19] = MASKCOL
        out[17] = b0[idx]
        out[18] = b1[idx]
        return out
    W["dxc_if"] = stack_xc_dec(FI, True)
    W["dxc_go"] = stack_xc_dec(OG, False)
    W["d0e_if"] = Wih0[FI, 0:8].T.copy()  # [8,128]
    W["d0e_go"] = Wih0[OG, 0:8].T.copy()
    W["wdT"] = Wd.T.copy()  # [64,8]
    W["bd"] = bd.reshape(8, 1).copy()
    for k in W:
        if k == "bd":
            W[k] = np.ascontiguousarray(W[k], np.float32)
        else:
            W[k] = np.ascontiguousarray(W[k]).astype(BF)
    return W


def build_streams(x, lengths_x, context, lengths_ctx):
    """Per-core streams. x [B,TW,16], context [B,TC,8]. B arbitrary.

    xc stream per tick: [20, 2B]: rows 0:16|16|17 x-part/mask/bias at cols
    0:B (zeros at B:2B); rows 18|19 l1 bias/mask at cols B:2B (zeros 0:B).
    """
    B = x.shape[0]
    TW = x.shape[1]
    TC = context.shape[1]
    SW, SD = 528, 512
    CW, CD = SW // 8, SD // 8

    s_idx = np.arange(SW)
    mw = (s_idx[:, None] < lengths_x[None, :]).astype(np.float32)  # [SW,B]
    mw1 = np.zeros_like(mw)
    mw1[1:] = mw[:-1]

    XC = np.zeros((CW, 20, 8, 2 * B), np.float32)
    xt = np.transpose(x, (1, 2, 0))  # [TW,16,B]
    for c in range(CW):
        for t in range(8):
            s = c * 8 + t
            if s < TW:
                XC[c, 0:16, t, 0:B] = xt[s]
            XC[c, 16, t, 0:B] = 1.0 - mw[s]
            XC[c, 17, t, 0:B] = 1.0
            XC[c, 18, t, B:2 * B] = 1.0
            XC[c, 19, t, B:2 * B] = 1.0 - mw1[s]
    NMw = np.zeros((CW, 128, 8, B), np.uint8)
    NMw[:, 0:64] = mw.reshape(CW, 8, 1, B).transpose(0, 2, 1, 3)
    NMw[:, 64:128] = mw1.reshape(CW, 8, 1, B).transpose(0, 2, 1, 3)

    md0 = np.zeros((SD,), np.float32)
    md0[0:TC - 1] = 1.0     # l0 steps 0..510 active; 511 pad frozen
    md1 = np.ones((SD,), np.float32)
    md1[0] = 0.0            # freeze l1 at slot 0
    DXC = np.zeros((CD, 20, 8, 2 * B), np.float32)
    ctxt = np.transpose(context, (1, 2, 0))  # [TC,8,B]
    for c in range(CD):
        for t in range(8):
            s = c * 8 + t
            if s < TC - 1:
                DXC[c, 0:8, t, 0:B] = ctxt[s]
            DXC[c, 16, t, 0:B] = 1.0 - md0[s]
            DXC[c, 17, t, 0:B] = 1.0
            DXC[c, 18, t, B:2 * B] = 1.0
            DXC[c, 19, t, B:2 * B] = 1.0 - md1[s]

    def pad1(a):
        return np.concatenate([a, np.zeros_like(a[:1])], 0)
    return dict(
        xc=pad1(XC.reshape(CW, 20, 16 * B)).astype(BF),
        nmw=pad1(NMw.reshape(CW, 128, 8 * B)),
        dxc=pad1(DXC.reshape(CD, 20, 16 * B)).astype(BF),
    )


def post_outputs(YE, YD, bd, lengths_ctx, TC):
    """YE [8,B], YD [CD,8,8,B] -> out [B,TC,8] with -999 padding."""
    B = YE.shape[1]
    out = np.zeros((B, TC, F), np.float32)
    out[:, 0, :] = YE.astype(np.float32).T
    ysd = YD.transpose(0, 2, 1, 3).reshape(512, F, B)  # [slot, F, B]
    # ys_t = slot t+1 for t = 0..510
    out[:, 1:TC, :] = ysd[1:TC].transpose(2, 0, 1) + bd[None, None, :]
    valid = np.arange(TC)[None, :] < lengths_ctx[:, None]
    return np.where(valid[:, :, None], out, np.float32(-999.0))


F32 = mybir.dt.float32
BF16 = mybir.dt.bfloat16
U8 = mybir.dt.uint8
AF = mybir.ActivationFunctionType

B = 64
H = 64
SW = 528   # warmup slots (padded; needs >= 513)
SD = 512   # decode slots (l0 steps 0..510 + 1 pad)
CW = SW // 8
CD = SD // 8


def build_nc(repeat=1, static=False):
    nc = bass.Bass("TRN2", target_bir_lowering=False, debug=False)
    dt = BF16

    d = {}
    d["xc"] = nc.dram_tensor("xc", [CW + 1, 20, 1024], dt, kind="ExternalInput")
    d["nmw"] = nc.dram_tensor("nmw", [CW + 1, 128, 512], U8, kind="ExternalInput")
    d["dxc"] = nc.dram_tensor("dxc", [CD + 1, 20, 1024], dt, kind="ExternalInput")
    for name, shp in [
        ("xc_if", [20, 128]), ("xc_go", [20, 128]),
        ("w0h_if", [64, 128]), ("w0h_go", [64, 128]),
        ("w1_if", [128, 128]), ("w1_go", [128, 128]),
        ("dxc_if", [20, 128]), ("dxc_go", [20, 128]),
        ("d0e_if", [8, 128]), ("d0e_go", [8, 128]),
        ("wdT", [64, 8]),
    ]:
        d[name] = nc.dram_tensor(name, shp, dt, kind="ExternalInput")
    d["bd"] = nc.dram_tensor("bd", [8, 1], F32, kind="ExternalInput")
    ye = nc.dram_tensor("ye", [8, B], dt, kind="ExternalOutput")
    yd = nc.dram_tensor("yd", [CD, 8, 512], F32, kind="ExternalOutput")

    with tile.TileContext(nc) as tc:
        with (
            tc.tile_pool(name="consts", bufs=1) as consts,
            tc.tile_pool(name="state", bufs=1) as state,
            tc.tile_pool(name="stream", bufs=1) as stream,
            tc.tile_pool(name="work", bufs=3) as work,
            tc.tile_pool(name="psum", bufs=2, space="PSUM") as psum,
            tc.tile_pool(name="psum1", bufs=1, space="PSUM") as psum1,
            tc.tile_pool(name="outp", bufs=1, space="PSUM") as outp,
        ):
            W = {}
            for name in ["xc_if", "xc_go", "w0h_if", "w0h_go", "w1_if",
                         "w1_go", "dxc_if", "dxc_go", "d0e_if", "d0e_go"]:
                t = consts.tile(list(d[name].shape), dt, tag=name, name="w_" + name)
                nc.sync.dma_start(out=t, in_=d[name][:, :])
                W[name] = t
            bd_t = consts.tile([8, 1], F32, tag="bd", name="w_bd")
            nc.sync.dma_start(out=bd_t, in_=d["bd"][:, :])
            W["bd"] = bd_t
            wdT_t = consts.tile([128, 8], dt, tag="wdT", name="w_wdT")
            nc.sync.dma_start(out=wdT_t[64:128, :], in_=d["wdT"][:, :])
            W["wdT"] = wdT_t

            rb = [state.tile([128, B], dt, tag=f"rb{i}", name=f"rb{i}") for i in range(2)]
            # U tiles: rows 0:64 = c state [c_l0 | c_l1], rows 64:128 = tanh(g) scratch
            uu = [state.tile([128, 2 * B], F32, tag=f"uu{i}", name=f"uu{i}") for i in range(2)]
            for i in range(2):
                nc.vector.memset(rb[i], 0.0)
                nc.vector.memset(uu[i], 0.0)

            saA = stream.tile([20, 1024], dt, tag="saA")
            saB = stream.tile([20, 1024], dt, tag="saB")
            nmA = stream.tile([128, 512], U8, tag="nmA")
            nmB = stream.tile([128, 512], U8, tag="nmB")
            elem = state.tile([8, B], dt, tag="elem")
            hkeep = state.tile([128, B], dt, tag="hkeep")
            nc.vector.memset(hkeep, 0.0)

            def tick(sl, sa, nm, decode):
                par = sl % 2
                rbp, rbn = rb[par], rb[1 - par]
                Up, Un = uu[par], uu[1 - par]
                t8 = sl % 8
                col2B = slice(t8 * 2 * B, (t8 + 1) * 2 * B)
                mega = psum.tile([128, 4 * B], F32, tag="mega", name="mega")

                wx_if = W["dxc_if"] if decode else W["xc_if"]
                wx_go = W["dxc_go"] if decode else W["xc_go"]

                # one accumulation group over the whole bank; stream-dependent
                # matmuls lead so the PE runs ahead of the recurrence
                nc.tensor.matmul(mega[:, 0:2 * B], wx_if, sa[0:20, col2B], start=True, stop=False)
                nc.tensor.matmul(mega[:, 2 * B:4 * B], wx_go, sa[0:20, col2B], start=False, stop=False)
                if decode:
                    nc.tensor.matmul(mega[:, 0:B], W["d0e_if"], elem, start=False, stop=False)
                    nc.tensor.matmul(mega[:, 2 * B:3 * B], W["d0e_go"], elem, start=False, stop=False)
                # h-dependent tail
                nc.tensor.matmul(mega[:, 0:B], W["w0h_if"], rbp[0:64, :], start=False, stop=False)
                nc.tensor.matmul(mega[:, B:2 * B], W["w1_if"], rbp[:, :], start=False, stop=False)
                nc.tensor.matmul(mega[:, 2 * B:3 * B], W["w0h_go"], rbp[0:64, :], start=False, stop=False)
                nc.tensor.matmul(mega[:, 3 * B:4 * B], W["w1_go"], rbp[:, :], start=False, stop=True)

                # activations: sif = sigmoid([f;i]); tanh(g) -> Up[64:128]
                sif = psum1.tile([128, 2 * B], F32, tag="sif", name="sif")
                so = work.tile([64, 2 * B], F32, tag="so", name="so")
                nc.scalar.activation(sif, mega[:, 0:2 * B], AF.Sigmoid)
                nc.scalar.activation(Up[64:128, :], mega[64:128, 2 * B:4 * B], AF.Tanh)
                nc.scalar.activation(so, mega[0:64, 2 * B:4 * B], AF.Sigmoid)

                # c/h update: V = [f*c ; i*g~] (PSUM so the fold is legal)
                V = psum1.tile([128, 2 * B], F32, tag="V", name="V")
                th = work.tile([64, 2 * B], F32, tag="th", name="th")
                nc.vector.tensor_mul(V, sif, Up)
                nc.vector.tensor_add(Un[0:64, :], V[0:64, :], V[64:128, :])
                nc.scalar.activation(th, Un[0:64, :], AF.Tanh)
                nc.vector.tensor_mul(rbn[0:64, :], so[:, 0:B], th[:, 0:B])
                nc.vector.tensor_mul(rbn[64:128, :], so[:, B:2 * B], th[:, B:2 * B])

                if nm is not None:
                    # capture h at each sample's last active slot (off the
                    # recurrence critical path)
                    nc.vector.copy_predicated(hkeep, nm[:, slice(t8 * B, (t8 + 1) * B)], rbn)
                return rbn

            rep_cm = tc.For_i(0, repeat, 1) if repeat > 1 else contextlib.nullcontext()
            with rep_cm:
                # ================= warmup =================
                nc.sync.dma_start(out=saA, in_=d["xc"][0, :, :])
                nc.sync.dma_start(out=nmA, in_=d["nmw"][0, :, :])
                def warm_body(j, i1, i2, first=False):
                    nc.sync.dma_start(out=saB, in_=d["xc"][i1, :, :])
                    nc.sync.dma_start(out=nmB, in_=d["nmw"][i1, :, :])
                    for sl in range(8):
                        tick(sl, saA, nmA, False)
                        if first and sl == 0:
                            nc.vector.memset(rb[1][64:128, :], 0.0)
                    nc.sync.dma_start(out=saA, in_=d["xc"][i2, :, :])
                    nc.sync.dma_start(out=nmA, in_=d["nmw"][i2, :, :])
                    for sl in range(8, 16):
                        tick(sl, saB, nmB, False)

                if static:
                    for j in range(CW // 2):
                        warm_body(j, j * 2 + 1, j * 2 + 2, first=(j == 0))
                else:
                    warm_body(0, 1, 2, first=True)
                    with tc.For_i(1, CW // 2, 1, hint_engines=(mybir.EngineType.PE,)) as j:
                        warm_body(j, nc.snap(j * 2 + 1), nc.snap(j * 2 + 2))

                # ================= elem =================
                nc.vector.tensor_copy(rb[0], hkeep)
                pe = outp.tile([8, B], F32, tag="ops", name="pe")
                nc.tensor.matmul(pe, W["wdT"][64:128, :], rb[0][64:128, :], start=True, stop=True)
                nc.scalar.activation(elem, pe, AF.Identity, bias=W["bd"][:, 0:1])
                nc.sync.dma_start(out=ye[:, :], in_=elem)

                # ================= decode =================
                nc.sync.dma_start(out=saA, in_=d["dxc"][0, :, :])
                def dec_body(j, i0, i1, i2, first=False):
                    nc.sync.dma_start(out=saB, in_=d["dxc"][i1, :, :])
                    ops = outp.tile([8, 512], F32, tag="ops", name="ops")
                    for sl in range(8):
                        rbn = tick(sl, saA, None, True)
                        if first and sl == 0:
                            nc.vector.tensor_copy(rb[1][64:128, :], rb[0][64:128, :])
                        nc.tensor.matmul(ops[:, sl * B:(sl + 1) * B], W["wdT"][64:128, :],
                                         rbn[64:128, :], start=True, stop=True)
                    oso = work.tile([8, 512], F32, tag="oso", name="oso")
                    nc.scalar.copy(oso, ops)
                    nc.sync.dma_start(out=yd[i0, :, :], in_=oso)
                    nc.sync.dma_start(out=saA, in_=d["dxc"][i2, :, :])
                    ops2 = outp.tile([8, 512], F32, tag="ops2", name="ops2")
                    for sl in range(8, 16):
                        rbn = tick(sl, saB, None, True)
                        nc.tensor.matmul(ops2[:, (sl - 8) * B:(sl - 7) * B], W["wdT"][64:128, :],
                                         rbn[64:128, :], start=True, stop=True)
                    oso2 = work.tile([8, 512], F32, tag="oso2", name="oso2")
                    nc.scalar.copy(oso2, ops2)
                    nc.sync.dma_start(out=yd[i1, :, :], in_=oso2)

                if static:
                    for j in range(CD // 2):
                        dec_body(j, j * 2, j * 2 + 1, j * 2 + 2, first=(j == 0))
                else:
                    dec_body(0, 0, 1, 2, first=True)
                    with tc.For_i(1, CD // 2, 1, hint_engines=(mybir.EngineType.PE,)) as j:
                        dec_body(j, nc.snap(j * 2), nc.snap(j * 2 + 1), nc.snap(j * 2 + 2))

    return nc


def legalize_waits(nc, max_waits=1):
    """walrus codegen caps semaphore waits per instruction; move extras onto
    NoOp instructions inserted immediately before (same engine)."""
    j = json.loads(mybir.module_to_json_bytes(nc.m))
    for fn in j.get("functions", []):
        for blk in fn.get("blocks", []):
            out = []
            for inst in blk.get("instructions", []):
                si = inst.get("sync_info") or {}
                waits = si.get("on_wait") or []
                if len(waits) > max_waits:
                    keep, extra = waits[-max_waits:], waits[:-max_waits]
                    for k, w in enumerate(extra):
                        out.append({"name": f"{inst['name']}-wsp{k}",
                                    "opcode": "NoOp", "engine": inst["engine"],
                                    "ins": [], "outs": [],
                                    "sync_info": {"on_wait": [w], "on_update": []}})
                    si = dict(si); si["on_wait"] = keep
                    inst = dict(inst); inst["sync_info"] = si
                out.append(inst)
            blk["instructions"] = out
    nc.m = mybir.module_from_json_bytes(json.dumps(j).encode())
    return nc


_NC_CACHE = {}


def _get_nc(repeat=1):
    if repeat not in _NC_CACHE:
        nc = build_nc(repeat)
        legalize_waits(nc)
        _NC_CACHE[repeat] = nc
    return _NC_CACHE[repeat]


def build_in_maps(x, lengths_x, context, lengths_ctx,
                  Wih0, Whh0, bih0, bhh0, Wih1, Whh1, bih1, bhh1, Wd, bd):
    Wt = build_weights(Wih0.astype(np.float32), Whh0.astype(np.float32),
                       bih0.astype(np.float32), bhh0.astype(np.float32),
                       Wih1.astype(np.float32), Whh1.astype(np.float32),
                       bih1.astype(np.float32), bhh1.astype(np.float32),
                       Wd.astype(np.float32), bd.astype(np.float32))
    Bn = x.shape[0] // N_CORES
    in_maps = []
    for core in range(N_CORES):
        sl = slice(core * Bn, (core + 1) * Bn)
        st = build_streams(np.ascontiguousarray(x[sl], dtype=np.float32),
                           np.asarray(lengths_x[sl], dtype=np.int64),
                           np.ascontiguousarray(context[sl], dtype=np.float32),
                           np.asarray(lengths_ctx[sl], dtype=np.int64))
        m = dict(st)
        m.update(Wt)
        in_maps.append(m)
    return in_maps


def kernel(x, lengths_x, context, lengths_ctx,
           Wih0, Whh0, bih0, bhh0, Wih1, Whh1, bih1, bhh1, Wd, bd):
    x = np.asarray(x)
    context = np.asarray(context)
    lengths_x = np.asarray(lengths_x)
    lengths_ctx = np.asarray(lengths_ctx)
    in_maps = build_in_maps(x, lengths_x, context, lengths_ctx,
                            np.asarray(Wih0), np.asarray(Whh0), np.asarray(bih0),
                            np.asarray(bhh0), np.asarray(Wih1), np.asarray(Whh1),
                            np.asarray(bih1), np.asarray(bhh1), np.asarray(Wd),
                            np.asarray(bd))
    nc = _get_nc(1)
    res = run_bass_kernel_spmd(nc, in_maps, core_ids=list(range(N_CORES)))
    Bn = x.shape[0] // N_CORES
    outs = []
    bd32 = np.asarray(bd, dtype=np.float32)
    for core in range(N_CORES):
        sl = slice(core * Bn, (core + 1) * Bn)
        YE = res.results[core]["ye"]
        YD = res.results[core]["yd"].reshape(CD, 8, 8, Bn)
        outs.append(post_outputs(YE, YD, bd32,
                                 np.asarray(lengths_ctx[sl], dtype=np.int64), TC))
    return np.concatenate(outs, axis=0).astype(np.float32)
